# revision 27
# baseline (speedup 1.0000x reference)
"""BertBidaf attention-flow kernel for 8 TRN2 NeuronCores — v12 (hybrid).

Sharding: data-parallel over batch (B=16 -> 2 batches per core); weights
replicated.

The device computes the attention-heavy ~98% of FLOPs: the trilinear
similarity matmul (with the c2q / c*c2q contraction terms riding as 128
extra rhs columns P0/P1), the row softmax statistics, and the fused
attention reductions for terms 2+3. The rank-1 projections
(cwc = c@w_c, q2c = b_att@c, c @ (W1 + W4*q2c)), the softmax division,
and the final row masking are host post-processing (~2% of FLOPs).

v12 over v10 (31.6us official / 34.6us traced):
  - batch 1's q ships RAW (262KB) instead of three pre-scaled copies
    (786KB); the DVE builds qws1 = q1 * {w_cq, W3_0, W3_1} during its
    idle early window.  Batch 0 stays pre-scaled: DVE broadcast
    multiplies run at 1 elem/cycle (stride-0 AP disables the 2x packed
    mode), too slow to feed the first matmuls.  DMA 4.74 -> 4.21MB.
  - cT is tile-major in DRAM; all large inputs move as ~131-393KB
    half-tile pieces alternating between the two HWDGE queues in PE
    consumption order, so the PE trails the DMA wavefront by <1 piece.
  - ~40 warm-up matmuls on a scratch PSUM bank bridge the DMA lead-in
    so the HAM clock gate (cold 1.2GHz) flips to 2.4GHz before the
    real matmuls start; PE idle gaps stay under the ~3.4us MID window.
  - bias matmuls OPEN each accumulation group (start=True) so they run
    early, off the critical tail.
  - exp emits den via activation accum_out (ACTIVATE +
    ACTIVATION_READ_ACCUMULATOR), dropping one DVE reduce per tile.
    (tensor_tensor_reduce wedges the HW runtime — do not use it.)

Per-batch device math (unchanged):
  ps[t] [128, 192] = rank-3 bias matmul (q-side biases + both sequence
  masks + q@W2+b_out on the P columns) + 16 accumulating chunk matmuls
  over d; then nrm = -rowmax(s); e = exp(s+nrm), den = sum(e);
  t23raw[k] = rowdot(e, P_k); [t23raw | nrm | den] -> outv[b].
Host post: m = c@w_c - nrm; b_att = softmax(m); q2c = b_att @ c;
  out = c @ (W1 + W4*q2c) + t23raw/den ; masked rows -> -1e12.
"""

import numpy as np
import ml_dtypes

B, C, Q, D = 16, 384, 64, 2048
NCORES = 8
BPC = B // NCORES  # batches per core
NCH = D // 128     # 16 d-chunks
NW = 192           # rhs width: 64 s-cols + 2x64 P-cols (c2q/c*c2q)
NEG = np.float32(-1e12)
BF16 = ml_dtypes.bfloat16

_cache = {}


def _build_nc():
    import concourse.bass as bass
    import concourse.bacc as bacc
    from concourse import mybir

    f32 = mybir.dt.float32
    bf16 = mybir.dt.bfloat16
    Ax = mybir.AxisListType.X
    Exp = mybir.ActivationFunctionType.Exp
    mul_op = mybir.AluOpType.mult
    add_op = mybir.AluOpType.add
    max_op = mybir.AluOpType.max

    nc = bacc.Bacc("TRN2", target_bir_lowering=False, debug=False)

    cT = nc.declare_dram_parameter("cT", [BPC, 128, 3, NCH, 128], bf16,
                                   isOutput=False)
    qwx0 = nc.declare_dram_parameter("qwx0", [128, NCH, NW], bf16,
                                     isOutput=False)
    qr1 = nc.declare_dram_parameter("qr1", [128, NCH, Q], bf16,
                                    isOutput=False)
    wv = nc.declare_dram_parameter("wv", [128, NCH, 3], bf16,
                                   isOutput=False)
    bias2 = nc.declare_dram_parameter("bias2", [3, BPC, NW + C], bf16,
                                      isOutput=False)
    outv = nc.declare_dram_parameter("outv", [BPC, 128, 12], f32,
                                     isOutput=True)

    b2s = nc.alloc_sbuf_tensor("b2s", [3, BPC, NW + C], bf16)
    wvs = nc.alloc_sbuf_tensor("wvs", [128, NCH, 3], bf16)
    qraw1 = nc.alloc_sbuf_tensor("qraw1", [128, NCH, Q], bf16)
    qws = [nc.alloc_sbuf_tensor(f"qws{b}", [128, NCH, NW], bf16)
           for b in range(BPC)]
    cts = [nc.alloc_sbuf_tensor(f"cts{b}", [128, 3, NCH, 128], bf16)
           for b in range(BPC)]
    dum = nc.alloc_sbuf_tensor("dum", [128, 2, 128], bf16)
    es = nc.alloc_sbuf_tensor("es", [128, 2 * 3, Q], f32)
    scr = nc.alloc_sbuf_tensor("scr", [128, 6, 2, Q], f32)
    ovs = [nc.alloc_sbuf_tensor(f"ov{b}", [128, 3, 4], f32)
           for b in range(BPC)]
    ps = [[nc.alloc_psum_tensor(f"ps{b}{t}", [128, NW], f32)
           for t in range(3)] for b in range(BPC)]
    pw = nc.alloc_psum_tensor("pw", [128, 256], f32)

    # one semaphore per input DMA (completions across queues are unordered)
    gb = nc.alloc_semaphore("gb")      # bias2
    gw = nc.alloc_semaphore("gw")      # wv
    w0a = nc.alloc_semaphore("w0a")    # qwx0 chs 0:8
    w0b = nc.alloc_semaphore("w0b")    # qwx0 chs 8:16
    q1a = nc.alloc_semaphore("q1a")    # qr1 chs 0:8
    q1b = nc.alloc_semaphore("q1b")    # qr1 chs 8:16
    # cts piece sems: cs[b][t][half]
    cs = [[[nc.alloc_semaphore(f"c{b}{t}{h}") for h in range(2)]
           for t in range(3)] for b in range(BPC)]
    # b1t2 moves as four quarters (last tile: smallest possible tail)
    u2q = [nc.alloc_semaphore(f"u2q{i}") for i in range(4)]
    dsem = nc.alloc_semaphore("dsem")  # dum scratch initialized
    bsem = nc.alloc_semaphore("bsem")  # DVE qws1 builds
    pesem = nc.alloc_semaphore("pesem")  # PE per-tile accumulation stops
    nsem = nc.alloc_semaphore("nsem")    # DVE nrm writes
    scsem = nc.alloc_semaphore("scsem")  # Scalar exp (+den accum) done
    xsem = nc.alloc_semaphore("xsem")    # DVE scr write -> reduce RAW
    dvsem = nc.alloc_semaphore("dvsem")  # DVE per-tile epilogue complete
    osem = nc.alloc_semaphore("osem")    # output DMA completions

    gscr = nc.alloc_sbuf_tensor("gscr", [128, 8, NW], bf16)
    gpr = nc.alloc_semaphore("gpr")    # gpsimd probe ops done

    TILES = [(b, t) for b in range(BPC) for t in range(3)]

    def ctp(b, t, lo, hi):
        return (cts[b][:, t, lo:hi, :], cT[b, :, t, lo:hi, :])

    with nc.Block(name="kern", no_gpsimd_drain=True) as blk:

        @blk.gpsimd
        def _(eng):
            # throughput probe: measure Pool-engine tensor_tensor cost on
            # HW (same shape as a half-batch qws build) before moving the
            # real builds here; results land in scratch, nothing reads them
            eng.wait_ge(gw, 16)
            eng.wait_ge(q1a, 16)
            for k in range(3):
                wk = wvs[:, 0:8, k:k + 1]
                wb = bass.AP(tensor=wk.tensor, offset=wk.offset,
                             ap=[wk.ap[0], [3, 8], [0, Q]])
                eng.tensor_tensor(out=gscr[:, :, 64 * k:64 * (k + 1)],
                                  in0=qraw1[:, 0:8, :], in1=wb,
                                  op=mul_op).then_inc(gpr, 1)

        @blk.sync
        def _(eng):
            eng.dma_start(out=b2s[:, :, :], in_=bias2[:, :, :]) \
                .then_inc(gb, 16)
            for (o, i), sem in [
                (ctp(0, 0, 8, 16), cs[0][0][1]),
                (ctp(0, 0, 0, 8), cs[0][0][0]),
                (ctp(0, 1, 0, 8), cs[0][1][0]),
                (ctp(0, 2, 0, 8), cs[0][2][0]),
                (ctp(1, 0, 0, 8), cs[1][0][0]),
                (ctp(1, 1, 0, 8), cs[1][1][0]),
                (ctp(1, 1, 8, 16), cs[1][1][1]),
                (ctp(1, 2, 0, 4), u2q[0]),
                (ctp(1, 2, 4, 8), u2q[1]),
            ]:
                eng.dma_start(out=o, in_=i).then_inc(sem, 16)
            for b in range(BPC):
                eng.wait_ge(dvsem, 3 * (b + 1))
                eng.dma_start(out=outv[b, :, :],
                              in_=ovs[b].rearrange("p a b -> p (a b)")) \
                    .then_inc(osem, 16)
            eng.wait_ge(osem, 32)

        @blk.scalar
        def _(eng):
            eng.dma_start(out=wvs[:, :, :], in_=wv[:, :, :]) \
                .then_inc(gw, 16)
            eng.dma_start(out=qws[0][:, 8:16, :], in_=qwx0[:, 8:16, :]) \
                .then_inc(w0b, 16)
            eng.dma_start(out=qws[0][:, 0:8, :], in_=qwx0[:, 0:8, :]) \
                .then_inc(w0a, 16)
            eng.dma_start(out=qraw1[:, 0:8, :], in_=qr1[:, 0:8, :]) \
                .then_inc(q1a, 16)
            eng.dma_start(out=qraw1[:, 8:16, :], in_=qr1[:, 8:16, :]) \
                .then_inc(q1b, 16)
            for (o, i), sem in [
                (ctp(0, 1, 8, 16), cs[0][1][1]),
                (ctp(0, 2, 8, 16), cs[0][2][1]),
                (ctp(1, 0, 8, 16), cs[1][0][1]),
                (ctp(1, 2, 8, 12), u2q[2]),
                (ctp(1, 2, 12, 16), u2q[3]),
            ]:
                eng.dma_start(out=o, in_=i).then_inc(sem, 16)
            for i, (b, t) in enumerate(TILES):
                eng.wait_ge(nsem, i + 1)
                eng.activation(es[:, i, :], ps[b][t][:, 0:Q], Exp,
                               bias=ovs[b][:, t, 2:3], scale=1.0,
                               accum_out=ovs[b][:, t, 3:4]) \
                    .then_inc(scsem, 1)

        @blk.tensor
        def _(eng):
            # warm-up matmuls on scratch data: keep the PE busy through
            # the DMA lead-in so the HAM clock-gate is at 2.4GHz when
            # the real matmuls start
            def dummy(n):
                for _ in range(n):
                    eng.matmul(pw[:, 0:128], dum[:, 0, :], dum[:, 1, :],
                               start=True, stop=True)

            def dummy2(n):
                # 256-col warm-phase dummies: more time per instruction
                wide = dum.rearrange("p a b -> p (a b)")
                for _ in range(n):
                    eng.matmul(pw[:, :], dum[:, 0, :], wide,
                               start=True, stop=True)

            eng.wait_ge(dsem, 1)
            dummy(28)
            # bias matmuls OPEN each accumulation group (start=True) so
            # they run early, off the critical tail
            eng.wait_ge(gb, 16)
            for b, t in TILES:
                eng.matmul(ps[b][t][:, :],
                           b2s[:, b, NW + 128 * t:NW + 128 * (t + 1)],
                           b2s[:, b, 0:NW], start=True, stop=False)
            # keep the PE busy until the DMA wavefront is ~2 pieces ahead:
            # a HAM MID window fires on sub-window idle, so the real MM
            # stream must never wait on a piece semaphore
            dummy2(10)

            def grp(b, t, lo, hi, close, waits):
                for w in waits:
                    eng.wait_ge(w, 16)
                for ch in range(lo, hi):
                    mm = eng.matmul(ps[b][t][:, :],
                                    cts[b][:, t, ch, :],
                                    qws[b][:, ch, :],
                                    start=False, stop=(close and ch == hi - 1))
                    if close and ch == hi - 1:
                        mm.then_inc(pesem, 1)

            # piece order = expected DMA arrival order; each tile's
            # last-arriving piece closes its accumulation group
            grp(0, 0, 8, 16, False, [w0b, cs[0][0][1]])
            grp(0, 0, 0, 8, True, [w0a, cs[0][0][0]])
            grp(0, 1, 0, 8, False, [cs[0][1][0]])
            grp(0, 1, 8, 16, True, [cs[0][1][1]])
            grp(0, 2, 0, 8, False, [cs[0][2][0]])
            grp(0, 2, 8, 16, True, [cs[0][2][1]])
            eng.wait_ge(bsem, 3)
            grp(1, 0, 0, 8, False, [cs[1][0][0]])
            eng.wait_ge(bsem, 6)
            grp(1, 0, 8, 16, True, [cs[1][0][1]])
            grp(1, 1, 0, 8, False, [cs[1][1][0]])
            grp(1, 1, 8, 16, True, [cs[1][1][1]])
            grp(1, 2, 8, 12, False, [u2q[2]])
            grp(1, 2, 12, 16, False, [u2q[3]])
            grp(1, 2, 0, 4, False, [u2q[0]])
            grp(1, 2, 4, 8, True, [u2q[1]])

        @blk.vector
        def _(eng):
            eng.memset(dum[:, :, :], 0.5).then_inc(dsem, 1)

            # build qws1[:, ch, 64k:64k+64] = qraw1[:, ch, :] * wv[:, ch, k]
            # (wv broadcast over the 64 q-cols via a stride-0 AP axis);
            # halves so the PE's b1t0 chunk groups unblock sooner
            def build(lo, hi, k):
                wk = wvs[:, lo:hi, k:k + 1]
                wb = bass.AP(tensor=wk.tensor, offset=wk.offset,
                             ap=[wk.ap[0], [3, hi - lo], [0, Q]])
                eng.tensor_tensor(out=qws[1][:, lo:hi, 64 * k:64 * (k + 1)],
                                  in0=qraw1[:, lo:hi, :], in1=wb,
                                  op=mul_op).then_inc(bsem, 1)

            eng.wait_ge(gw, 16)
            eng.wait_ge(q1a, 16)
            for k in range(3):
                build(0, 8, k)
            eng.wait_ge(q1b, 16)
            for k in range(3):
                build(8, 16, k)

            # software-pipelined epilogue: tile i+1's nrm issues before
            # tile i's mul/reduce so the Scalar exp overlaps the DVE work
            def nrm(i):
                b, t = TILES[i]
                eng.wait_ge(pesem, i + 1)
                eng.tensor_reduce(out=ovs[b][:, t, 2:3],
                                  in_=ps[b][t][:, 0:Q], axis=Ax,
                                  op=max_op, negate=True).then_inc(nsem, 1)

            def body(i):
                b, t = TILES[i]
                eng.wait_ge(scsem, i + 1)
                e = es[:, i, :]
                e_dup = bass.AP(tensor=e.tensor, offset=e.offset,
                                ap=[e.ap[0], [0, 2], e.ap[1]])
                eng.tensor_tensor(
                    out=scr[:, i, :, :],
                    in0=ps[b][t][:, Q:3 * Q].rearrange("p (j i) -> p j i",
                                                       j=2),
                    in1=e_dup, op=mul_op).then_inc(xsem, 1)
                eng.wait_ge(xsem, i + 1)
                eng.tensor_reduce(out=ovs[b][:, t, 0:2],
                                  in_=scr[:, i, :, :],
                                  axis=Ax, op=add_op).then_inc(dvsem, 1)

            nrm(0)
            for i in range(1, 6):
                nrm(i)
                body(i - 1)
            body(5)

    nc.finalize()
    return nc


def _get_nc():
    if "nc" not in _cache:
        _cache["nc"] = _build_nc()
    return _cache["nc"]


def _prep_host(c, q, c_len, q_len, w_c, b_c, w_q, b_q, w_cq, b_cq, W_out,
               b_out):
    """Build per-core device input maps (host-side layout/masking prep)."""
    c = np.asarray(c, np.float32)
    q = np.asarray(q, np.float32)
    c_len = np.asarray(c_len).astype(np.int64)
    q_len = np.asarray(q_len).astype(np.int64)
    w_c = np.asarray(w_c, np.float32)
    w_q = np.asarray(w_q, np.float32)
    w_cq = np.asarray(w_cq, np.float32)
    W_out = np.asarray(W_out, np.float32)
    b_out = np.asarray(b_out, np.float32)
    b_sum = float(np.asarray(b_c, np.float32) + np.asarray(b_q, np.float32)
                  + np.asarray(b_cq, np.float32))

    Mv = np.float32(BF16(-1e12))
    iq = np.arange(Q)
    W2 = W_out[D:2 * D]       # [D, 2] (x = [c, c2q, c*c2q, c*q2c])
    W3 = W_out[2 * D:3 * D]

    wvm = np.stack([w_cq, W3[:, 0], W3[:, 1]], axis=1) \
        .reshape(NCH, 128, 3).transpose(1, 0, 2).astype(BF16)

    in_maps = []
    for core in range(NCORES):
        bs = [BPC * core + i for i in range(BPC)]
        cTm = np.empty((BPC, 128, 3, NCH, 128), BF16)
        b2 = np.zeros((3, BPC, NW + C), BF16)
        for i, bidx in enumerate(bs):
            cTm[i] = c[bidx].T.reshape(NCH, 128, 3, 128) \
                .transpose(1, 2, 0, 3).astype(BF16)
            qb = q[bidx]
            qs = qb @ w_q + b_sum
            low = np.where(iq >= q_len[bidx], Mv, np.float32(0))
            hi = np.where((iq < Q - 1) | (iq >= q_len[bidx]), Mv,
                          np.float32(0))
            QW2b = qb @ W2 + b_out[None, :]
            b2[0, i, 0:64] = qs.astype(BF16)
            b2[0, i, 64:128] = QW2b[:, 0].astype(BF16)
            b2[0, i, 128:192] = QW2b[:, 1].astype(BF16)
            b2[1, i, 0:64] = low.astype(BF16)
            b2[2, i, 0:64] = (hi - low).astype(BF16)
            b2[0, i, NW:NW + C] = BF16(1)
            b2[1, i, NW:NW + C] = BF16(1)
            b2[2, i, NW:NW + C] = (np.arange(C) >= c_len[bidx]) \
                .astype(np.float32).astype(BF16)
        # batch 0: pre-scaled q-side rhs (3 copies)
        q0T = q[bs[0]].T                          # [D, Q]
        blk = np.empty((D, NW), np.float32)
        blk[:, 0:64] = q0T * w_cq[:, None]
        blk[:, 64:128] = q0T * W3[:, 0:1]
        blk[:, 128:192] = q0T * W3[:, 1:2]
        qwx0m = blk.reshape(NCH, 128, NW).transpose(1, 0, 2).astype(BF16)
        # batch 1: raw q, scaled on device
        qr1m = q[bs[1]].T.reshape(NCH, 128, Q).transpose(1, 0, 2) \
            .astype(BF16)
        in_maps.append(dict(cT=cTm, qwx0=qwx0m, qr1=qr1m, wv=wvm, bias2=b2))
    return in_maps, (c, c_len, W_out, w_c)


def kernel(**inputs):
    from concourse.bass_utils import run_bass_kernel_spmd

    nc = _get_nc()
    in_maps, (c, c_len, W_out, w_c) = _prep_host(**inputs)
    res = run_bass_kernel_spmd(nc, in_maps, core_ids=list(range(NCORES)))
    _cache["last_results"] = res

    W1 = W_out[0:D]          # [D, 2]
    W4 = W_out[3 * D:4 * D]

    out = np.empty((B, C, 2), np.float32)
    for core in range(NCORES):
        o = res.results[core]["outv"].reshape(BPC, 128, 3, 4)
        for i in range(BPC):
            bidx = BPC * core + i
            den = o[i, :, :, 3].T.reshape(C)
            t23 = o[i, :, :, 0:2].transpose(1, 0, 2).reshape(C, 2) \
                / den[:, None]
            nrm = o[i, :, :, 2].T.reshape(C)
            m = c[bidx] @ w_c - nrm
            eb = np.exp(m - m.max())
            b_att = (eb / eb.sum()).astype(np.float32)
            q2c = b_att @ c[bidx]                       # [D]
            w14 = W1 + W4 * q2c[:, None]                # [D, 2]
            out[bidx] = c[bidx] @ w14 + t23

    rows = np.arange(C)[None, :]
    row_mask = (rows >= c_len[:, None]) & (rows < C - 1)
    out0 = np.where(row_mask, NEG, out[..., 0])
    out1 = np.where(row_mask, NEG, out[..., 1])
    return out0, out1


# revision 28
# speedup vs baseline: 1.0725x; 1.0725x over previous
"""BertBidaf attention-flow kernel for 8 TRN2 NeuronCores — v12 (hybrid).

Sharding: data-parallel over batch (B=16 -> 2 batches per core); weights
replicated.

The device computes the attention-heavy ~98% of FLOPs: the trilinear
similarity matmul (with the c2q / c*c2q contraction terms riding as 128
extra rhs columns P0/P1), the row softmax statistics, and the fused
attention reductions for terms 2+3. The rank-1 projections
(cwc = c@w_c, q2c = b_att@c, c @ (W1 + W4*q2c)), the softmax division,
and the final row masking are host post-processing (~2% of FLOPs).

v12 over v10 (31.6us official / 34.6us traced):
  - batch 1's q ships RAW (262KB) instead of three pre-scaled copies
    (786KB); the DVE builds qws1 = q1 * {w_cq, W3_0, W3_1} during its
    idle early window.  Batch 0 stays pre-scaled: DVE broadcast
    multiplies run at 1 elem/cycle (stride-0 AP disables the 2x packed
    mode), too slow to feed the first matmuls.  DMA 4.74 -> 4.21MB.
  - cT is tile-major in DRAM; all large inputs move as ~131-393KB
    half-tile pieces alternating between the two HWDGE queues in PE
    consumption order, so the PE trails the DMA wavefront by <1 piece.
  - ~40 warm-up matmuls on a scratch PSUM bank bridge the DMA lead-in
    so the HAM clock gate (cold 1.2GHz) flips to 2.4GHz before the
    real matmuls start; PE idle gaps stay under the ~3.4us MID window.
  - bias matmuls OPEN each accumulation group (start=True) so they run
    early, off the critical tail.
  - exp emits den via activation accum_out (ACTIVATE +
    ACTIVATION_READ_ACCUMULATOR), dropping one DVE reduce per tile.
    (tensor_tensor_reduce wedges the HW runtime — do not use it.)

Per-batch device math (unchanged):
  ps[t] [128, 192] = rank-3 bias matmul (q-side biases + both sequence
  masks + q@W2+b_out on the P columns) + 16 accumulating chunk matmuls
  over d; then nrm = -rowmax(s); e = exp(s+nrm), den = sum(e);
  t23raw[k] = rowdot(e, P_k); [t23raw | nrm | den] -> outv[b].
Host post: m = c@w_c - nrm; b_att = softmax(m); q2c = b_att @ c;
  out = c @ (W1 + W4*q2c) + t23raw/den ; masked rows -> -1e12.
"""

import numpy as np
import ml_dtypes

B, C, Q, D = 16, 384, 64, 2048
NCORES = 8
BPC = B // NCORES  # batches per core
NCH = D // 128     # 16 d-chunks
NW = 192           # rhs width: 64 s-cols + 2x64 P-cols (c2q/c*c2q)
NEG = np.float32(-1e12)
BF16 = ml_dtypes.bfloat16

_cache = {}


def _build_nc():
    import concourse.bass as bass
    import concourse.bacc as bacc
    from concourse import mybir

    f32 = mybir.dt.float32
    bf16 = mybir.dt.bfloat16
    Ax = mybir.AxisListType.X
    Exp = mybir.ActivationFunctionType.Exp
    mul_op = mybir.AluOpType.mult
    add_op = mybir.AluOpType.add
    max_op = mybir.AluOpType.max

    nc = bacc.Bacc("TRN2", target_bir_lowering=False, debug=False)

    cT = nc.declare_dram_parameter("cT", [BPC, 128, 3, NCH, 128], bf16,
                                   isOutput=False)
    qwx0 = nc.declare_dram_parameter("qwx0", [128, NCH, NW], bf16,
                                     isOutput=False)
    qr1 = nc.declare_dram_parameter("qr1", [128, NCH, Q], bf16,
                                    isOutput=False)
    wv = nc.declare_dram_parameter("wv", [128, NCH, 3], bf16,
                                   isOutput=False)
    bias2 = nc.declare_dram_parameter("bias2", [3, BPC, NW + C], bf16,
                                      isOutput=False)
    outv = nc.declare_dram_parameter("outv", [BPC, 128, 12], f32,
                                     isOutput=True)

    b2s = nc.alloc_sbuf_tensor("b2s", [3, BPC, NW + C], bf16)
    wvs = nc.alloc_sbuf_tensor("wvs", [128, NCH, 3], bf16)
    qraw1 = nc.alloc_sbuf_tensor("qraw1", [128, NCH, Q], bf16)
    qws = [nc.alloc_sbuf_tensor(f"qws{b}", [128, NCH, NW], bf16)
           for b in range(BPC)]
    cts = [nc.alloc_sbuf_tensor(f"cts{b}", [128, 3, NCH, 128], bf16)
           for b in range(BPC)]
    dum = nc.alloc_sbuf_tensor("dum", [128, 2, 128], bf16)
    es = nc.alloc_sbuf_tensor("es", [128, 2 * 3, Q], f32)
    scr = nc.alloc_sbuf_tensor("scr", [128, 6, 2, Q], f32)
    ovs = [nc.alloc_sbuf_tensor(f"ov{b}", [128, 3, 4], f32)
           for b in range(BPC)]
    ps = [[nc.alloc_psum_tensor(f"ps{b}{t}", [128, NW], f32)
           for t in range(3)] for b in range(BPC)]
    pw = nc.alloc_psum_tensor("pw", [128, 128], f32)

    # one semaphore per input DMA (completions across queues are unordered)
    gb = nc.alloc_semaphore("gb")      # bias2
    gw = nc.alloc_semaphore("gw")      # wv
    w0a = nc.alloc_semaphore("w0a")    # qwx0 chs 0:8
    w0b = nc.alloc_semaphore("w0b")    # qwx0 chs 8:16
    q1s = nc.alloc_semaphore("q1s")    # qr1
    # cts piece sems: cs[b][t][half]
    cs = [[[nc.alloc_semaphore(f"c{b}{t}{h}") for h in range(2)]
           for t in range(3)] for b in range(BPC)]
    c5c = nc.alloc_semaphore("c5c")    # b1t2 chs 12:16 (last sliver)
    dsem = nc.alloc_semaphore("dsem")  # dum scratch initialized
    bsem = nc.alloc_semaphore("bsem")  # DVE qws1 builds
    pesem = nc.alloc_semaphore("pesem")  # PE per-tile accumulation stops
    nsem = nc.alloc_semaphore("nsem")    # DVE nrm writes
    scsem = nc.alloc_semaphore("scsem")  # Scalar exp (+den accum) done
    xsem = nc.alloc_semaphore("xsem")    # DVE scr write -> reduce RAW
    dvsem = nc.alloc_semaphore("dvsem")  # DVE per-tile epilogue complete
    osem = nc.alloc_semaphore("osem")    # output DMA completions

    TILES = [(b, t) for b in range(BPC) for t in range(3)]

    def ctp(b, t, lo, hi):
        return (cts[b][:, t, lo:hi, :], cT[b, :, t, lo:hi, :])

    with nc.Block(name="kern", no_gpsimd_drain=True) as blk:

        @blk.sync
        def _(eng):
            eng.dma_start(out=b2s[:, :, :], in_=bias2[:, :, :]) \
                .then_inc(gb, 16)
            eng.dma_start(out=qws[0][:, 0:8, :], in_=qwx0[:, 0:8, :]) \
                .then_inc(w0a, 16)
            for (o, i), sem in [
                (ctp(0, 0, 8, 16), cs[0][0][1]),
                (ctp(0, 1, 0, 8), cs[0][1][0]),
                (ctp(0, 2, 0, 8), cs[0][2][0]),
                (ctp(1, 0, 0, 8), cs[1][0][0]),
                (ctp(1, 1, 0, 8), cs[1][1][0]),
                (ctp(1, 2, 0, 8), cs[1][2][0]),
                (ctp(1, 2, 12, 16), c5c),
            ]:
                eng.dma_start(out=o, in_=i).then_inc(sem, 16)
            for b in range(BPC):
                eng.wait_ge(dvsem, 3 * (b + 1))
                eng.dma_start(out=outv[b, :, :],
                              in_=ovs[b].rearrange("p a b -> p (a b)")) \
                    .then_inc(osem, 16)
            eng.wait_ge(osem, 32)

        @blk.scalar
        def _(eng):
            eng.dma_start(out=wvs[:, :, :], in_=wv[:, :, :]) \
                .then_inc(gw, 16)
            eng.dma_start(out=qws[0][:, 8:16, :], in_=qwx0[:, 8:16, :]) \
                .then_inc(w0b, 16)
            eng.dma_start(out=qraw1[:, :, :], in_=qr1[:, :, :]) \
                .then_inc(q1s, 16)
            for (o, i), sem in [
                (ctp(0, 0, 0, 8), cs[0][0][0]),
                (ctp(0, 1, 8, 16), cs[0][1][1]),
                (ctp(0, 2, 8, 16), cs[0][2][1]),
                (ctp(1, 0, 8, 16), cs[1][0][1]),
                (ctp(1, 1, 8, 16), cs[1][1][1]),
                (ctp(1, 2, 8, 12), cs[1][2][1]),
            ]:
                eng.dma_start(out=o, in_=i).then_inc(sem, 16)
            for i, (b, t) in enumerate(TILES):
                eng.wait_ge(nsem, i + 1)
                eng.activation(es[:, i, :], ps[b][t][:, 0:Q], Exp,
                               bias=ovs[b][:, t, 2:3], scale=1.0,
                               accum_out=ovs[b][:, t, 3:4]) \
                    .then_inc(scsem, 1)

        @blk.tensor
        def _(eng):
            # warm-up matmuls on scratch data: keep the PE busy through
            # the DMA lead-in so the HAM clock-gate is at 2.4GHz when
            # the real matmuls start
            def dummy(n):
                for _ in range(n):
                    eng.matmul(pw[:, :], dum[:, 0, :], dum[:, 1, :],
                               start=True, stop=True)

            eng.wait_ge(dsem, 1)
            dummy(26)
            # bias matmuls OPEN each accumulation group (start=True) so
            # they run early, off the critical tail
            eng.wait_ge(gb, 16)
            for b, t in TILES:
                eng.matmul(ps[b][t][:, :],
                           b2s[:, b, NW + 128 * t:NW + 128 * (t + 1)],
                           b2s[:, b, 0:NW], start=True, stop=False)
            dummy(14)

            def grp(b, t, lo, hi, close, waits):
                for w in waits:
                    eng.wait_ge(w, 16)
                for ch in range(lo, hi):
                    mm = eng.matmul(ps[b][t][:, :],
                                    cts[b][:, t, ch, :],
                                    qws[b][:, ch, :],
                                    start=False, stop=(close and ch == hi - 1))
                    if close and ch == hi - 1:
                        mm.then_inc(pesem, 1)

            # piece order = expected DMA arrival order; each tile's
            # last-arriving piece closes its accumulation group
            grp(0, 0, 8, 16, False, [w0b, cs[0][0][1]])
            grp(0, 0, 0, 8, True, [w0a, cs[0][0][0]])
            grp(0, 1, 0, 8, False, [cs[0][1][0]])
            grp(0, 1, 8, 16, True, [cs[0][1][1]])
            grp(0, 2, 0, 8, False, [cs[0][2][0]])
            grp(0, 2, 8, 16, True, [cs[0][2][1]])
            eng.wait_ge(bsem, 3)
            grp(1, 0, 0, 8, False, [cs[1][0][0]])
            grp(1, 0, 8, 16, True, [cs[1][0][1]])
            grp(1, 1, 0, 8, False, [cs[1][1][0]])
            grp(1, 1, 8, 16, True, [cs[1][1][1]])
            grp(1, 2, 0, 8, False, [cs[1][2][0]])
            grp(1, 2, 8, 12, False, [cs[1][2][1]])
            grp(1, 2, 12, 16, True, [c5c])

        @blk.vector
        def _(eng):
            eng.memset(dum[:, :, :], 0.5).then_inc(dsem, 1)

            # build qws1[:, ch, 64k:64k+64] = qraw1[:, ch, :] * wv[:, ch, k]
            # (wv broadcast over the 64 q-cols via a stride-0 AP axis)
            eng.wait_ge(gw, 16)
            eng.wait_ge(q1s, 16)
            for k in range(3):
                wk = wvs[:, :, k:k + 1]
                wb = bass.AP(tensor=wk.tensor, offset=wk.offset,
                             ap=[wk.ap[0], [3, NCH], [0, Q]])
                eng.tensor_tensor(out=qws[1][:, :, 64 * k:64 * (k + 1)],
                                  in0=qraw1[:, :, :], in1=wb,
                                  op=mul_op).then_inc(bsem, 1)

            # software-pipelined epilogue: tile i+1's nrm issues before
            # tile i's mul/reduce so the Scalar exp overlaps the DVE work
            def nrm(i):
                b, t = TILES[i]
                eng.wait_ge(pesem, i + 1)
                eng.tensor_reduce(out=ovs[b][:, t, 2:3],
                                  in_=ps[b][t][:, 0:Q], axis=Ax,
                                  op=max_op, negate=True).then_inc(nsem, 1)

            def body(i):
                b, t = TILES[i]
                eng.wait_ge(scsem, i + 1)
                e = es[:, i, :]
                e_dup = bass.AP(tensor=e.tensor, offset=e.offset,
                                ap=[e.ap[0], [0, 2], e.ap[1]])
                eng.tensor_tensor(
                    out=scr[:, i, :, :],
                    in0=ps[b][t][:, Q:3 * Q].rearrange("p (j i) -> p j i",
                                                       j=2),
                    in1=e_dup, op=mul_op).then_inc(xsem, 1)
                eng.wait_ge(xsem, i + 1)
                eng.tensor_reduce(out=ovs[b][:, t, 0:2],
                                  in_=scr[:, i, :, :],
                                  axis=Ax, op=add_op).then_inc(dvsem, 1)

            nrm(0)
            for i in range(1, 6):
                nrm(i)
                body(i - 1)
            body(5)

    nc.finalize()
    return nc


def _get_nc():
    if "nc" not in _cache:
        _cache["nc"] = _build_nc()
    return _cache["nc"]


def _prep_host(c, q, c_len, q_len, w_c, b_c, w_q, b_q, w_cq, b_cq, W_out,
               b_out):
    """Build per-core device input maps (host-side layout/masking prep)."""
    c = np.asarray(c, np.float32)
    q = np.asarray(q, np.float32)
    c_len = np.asarray(c_len).astype(np.int64)
    q_len = np.asarray(q_len).astype(np.int64)
    w_c = np.asarray(w_c, np.float32)
    w_q = np.asarray(w_q, np.float32)
    w_cq = np.asarray(w_cq, np.float32)
    W_out = np.asarray(W_out, np.float32)
    b_out = np.asarray(b_out, np.float32)
    b_sum = float(np.asarray(b_c, np.float32) + np.asarray(b_q, np.float32)
                  + np.asarray(b_cq, np.float32))

    Mv = np.float32(BF16(-1e12))
    iq = np.arange(Q)
    W2 = W_out[D:2 * D]       # [D, 2] (x = [c, c2q, c*c2q, c*q2c])
    W3 = W_out[2 * D:3 * D]

    wvm = np.stack([w_cq, W3[:, 0], W3[:, 1]], axis=1) \
        .reshape(NCH, 128, 3).transpose(1, 0, 2).astype(BF16)

    in_maps = []
    for core in range(NCORES):
        bs = [BPC * core + i for i in range(BPC)]
        cTm = np.empty((BPC, 128, 3, NCH, 128), BF16)
        b2 = np.zeros((3, BPC, NW + C), BF16)
        for i, bidx in enumerate(bs):
            cTm[i] = c[bidx].T.reshape(NCH, 128, 3, 128) \
                .transpose(1, 2, 0, 3).astype(BF16)
            qb = q[bidx]
            qs = qb @ w_q + b_sum
            low = np.where(iq >= q_len[bidx], Mv, np.float32(0))
            hi = np.where((iq < Q - 1) | (iq >= q_len[bidx]), Mv,
                          np.float32(0))
            QW2b = qb @ W2 + b_out[None, :]
            b2[0, i, 0:64] = qs.astype(BF16)
            b2[0, i, 64:128] = QW2b[:, 0].astype(BF16)
            b2[0, i, 128:192] = QW2b[:, 1].astype(BF16)
            b2[1, i, 0:64] = low.astype(BF16)
            b2[2, i, 0:64] = (hi - low).astype(BF16)
            b2[0, i, NW:NW + C] = BF16(1)
            b2[1, i, NW:NW + C] = BF16(1)
            b2[2, i, NW:NW + C] = (np.arange(C) >= c_len[bidx]) \
                .astype(np.float32).astype(BF16)
        # batch 0: pre-scaled q-side rhs (3 copies)
        q0T = q[bs[0]].T                          # [D, Q]
        blk = np.empty((D, NW), np.float32)
        blk[:, 0:64] = q0T * w_cq[:, None]
        blk[:, 64:128] = q0T * W3[:, 0:1]
        blk[:, 128:192] = q0T * W3[:, 1:2]
        qwx0m = blk.reshape(NCH, 128, NW).transpose(1, 0, 2).astype(BF16)
        # batch 1: raw q, scaled on device
        qr1m = q[bs[1]].T.reshape(NCH, 128, Q).transpose(1, 0, 2) \
            .astype(BF16)
        in_maps.append(dict(cT=cTm, qwx0=qwx0m, qr1=qr1m, wv=wvm, bias2=b2))
    return in_maps, (c, c_len, W_out, w_c)


def kernel(**inputs):
    from concourse.bass_utils import run_bass_kernel_spmd

    nc = _get_nc()
    in_maps, (c, c_len, W_out, w_c) = _prep_host(**inputs)
    res = run_bass_kernel_spmd(nc, in_maps, core_ids=list(range(NCORES)))
    _cache["last_results"] = res

    W1 = W_out[0:D]          # [D, 2]
    W4 = W_out[3 * D:4 * D]

    out = np.empty((B, C, 2), np.float32)
    for core in range(NCORES):
        o = res.results[core]["outv"].reshape(BPC, 128, 3, 4)
        for i in range(BPC):
            bidx = BPC * core + i
            den = o[i, :, :, 3].T.reshape(C)
            t23 = o[i, :, :, 0:2].transpose(1, 0, 2).reshape(C, 2) \
                / den[:, None]
            nrm = o[i, :, :, 2].T.reshape(C)
            m = c[bidx] @ w_c - nrm
            eb = np.exp(m - m.max())
            b_att = (eb / eb.sum()).astype(np.float32)
            q2c = b_att @ c[bidx]                       # [D]
            w14 = W1 + W4 * q2c[:, None]                # [D, 2]
            out[bidx] = c[bidx] @ w14 + t23

    rows = np.arange(C)[None, :]
    row_mask = (rows >= c_len[:, None]) & (rows < C - 1)
    out0 = np.where(row_mask, NEG, out[..., 0])
    out1 = np.where(row_mask, NEG, out[..., 1])
    return out0, out1


# revision 30
# speedup vs baseline: 1.1275x; 1.0512x over previous
"""BertBidaf attention-flow kernel for 8 TRN2 NeuronCores — v12 (hybrid).

Sharding: data-parallel over batch (B=16 -> 2 batches per core); weights
replicated.

The device computes the attention-heavy ~98% of FLOPs: the trilinear
similarity matmul (with the c2q / c*c2q contraction terms riding as 128
extra rhs columns P0/P1), the row softmax statistics, and the fused
attention reductions for terms 2+3. The rank-1 projections
(cwc = c@w_c, q2c = b_att@c, c @ (W1 + W4*q2c)), the softmax division,
and the final row masking are host post-processing (~2% of FLOPs).

v12 over v10 (31.6us official / 34.6us traced):
  - batch 1's q ships RAW (262KB) instead of three pre-scaled copies
    (786KB); the DVE builds qws1 = q1 * {w_cq, W3_0, W3_1} during its
    idle early window.  Batch 0 stays pre-scaled: DVE broadcast
    multiplies run at 1 elem/cycle (stride-0 AP disables the 2x packed
    mode), too slow to feed the first matmuls.  DMA 4.74 -> 4.21MB.
  - cT is tile-major in DRAM; all large inputs move as ~131-393KB
    half-tile pieces alternating between the two HWDGE queues in PE
    consumption order, so the PE trails the DMA wavefront by <1 piece.
  - ~40 warm-up matmuls on a scratch PSUM bank bridge the DMA lead-in
    so the HAM clock gate (cold 1.2GHz) flips to 2.4GHz before the
    real matmuls start; PE idle gaps stay under the ~3.4us MID window.
  - bias matmuls OPEN each accumulation group (start=True) so they run
    early, off the critical tail.
  - exp emits den via activation accum_out (ACTIVATE +
    ACTIVATION_READ_ACCUMULATOR), dropping one DVE reduce per tile.
    (tensor_tensor_reduce wedges the HW runtime — do not use it.)

Per-batch device math (unchanged):
  ps[t] [128, 192] = rank-3 bias matmul (q-side biases + both sequence
  masks + q@W2+b_out on the P columns) + 16 accumulating chunk matmuls
  over d; then nrm = -rowmax(s); e = exp(s+nrm), den = sum(e);
  t23raw[k] = rowdot(e, P_k); [t23raw | nrm | den] -> outv[b].
Host post: m = c@w_c - nrm; b_att = softmax(m); q2c = b_att @ c;
  out = c @ (W1 + W4*q2c) + t23raw/den ; masked rows -> -1e12.
"""

import numpy as np
import ml_dtypes

B, C, Q, D = 16, 384, 64, 2048
NCORES = 8
BPC = B // NCORES  # batches per core
NCH = D // 128     # 16 d-chunks
NW = 192           # rhs width: 64 s-cols + 2x64 P-cols (c2q/c*c2q)
NEG = np.float32(-1e12)
BF16 = ml_dtypes.bfloat16

_cache = {}


def _build_nc():
    import concourse.bass as bass
    import concourse.bacc as bacc
    from concourse import mybir

    f32 = mybir.dt.float32
    bf16 = mybir.dt.bfloat16
    Ax = mybir.AxisListType.X
    Exp = mybir.ActivationFunctionType.Exp
    mul_op = mybir.AluOpType.mult
    add_op = mybir.AluOpType.add
    max_op = mybir.AluOpType.max

    nc = bacc.Bacc("TRN2", target_bir_lowering=False, debug=False)

    cT = nc.declare_dram_parameter("cT", [BPC, 128, 3, NCH, 128], bf16,
                                   isOutput=False)
    qwx0 = nc.declare_dram_parameter("qwx0", [128, NCH, NW], bf16,
                                     isOutput=False)
    qr1 = nc.declare_dram_parameter("qr1", [128, NCH, Q], bf16,
                                    isOutput=False)
    wv = nc.declare_dram_parameter("wv", [128, NCH, 3], bf16,
                                   isOutput=False)
    bias2 = nc.declare_dram_parameter("bias2", [3, BPC, NW + C], bf16,
                                      isOutput=False)
    outv = nc.declare_dram_parameter("outv", [BPC, 128, 12], f32,
                                     isOutput=True)

    b2s = nc.alloc_sbuf_tensor("b2s", [3, BPC, NW + C], bf16)
    wvs = nc.alloc_sbuf_tensor("wvs", [128, NCH, 3], bf16)
    qraw1 = nc.alloc_sbuf_tensor("qraw1", [128, NCH, Q], bf16)
    qws = [nc.alloc_sbuf_tensor(f"qws{b}", [128, NCH, NW], bf16)
           for b in range(BPC)]
    cts = [nc.alloc_sbuf_tensor(f"cts{b}", [128, 3, NCH, 128], bf16)
           for b in range(BPC)]
    dum = nc.alloc_sbuf_tensor("dum", [128, 2, 128], bf16)
    es = nc.alloc_sbuf_tensor("es", [128, 2 * 3, Q], f32)
    scr = nc.alloc_sbuf_tensor("scr", [128, 6, 2, Q], f32)
    ovs = [nc.alloc_sbuf_tensor(f"ov{b}", [128, 3, 4], f32)
           for b in range(BPC)]
    ps = [[nc.alloc_psum_tensor(f"ps{b}{t}", [128, NW], f32)
           for t in range(3)] for b in range(BPC)]
    pw = nc.alloc_psum_tensor("pw", [128, 128], f32)

    # one semaphore per input DMA (completions across queues are unordered)
    gb = nc.alloc_semaphore("gb")      # bias2
    gw = nc.alloc_semaphore("gw")      # wv
    w0a = nc.alloc_semaphore("w0a")    # qwx0 chs 0:8
    w0b = nc.alloc_semaphore("w0b")    # qwx0 chs 8:16
    q1s = nc.alloc_semaphore("q1s")    # qr1
    # cts piece sems: cs[b][t][half]
    cs = [[[nc.alloc_semaphore(f"c{b}{t}{h}") for h in range(2)]
           for t in range(3)] for b in range(BPC)]
    c5c = nc.alloc_semaphore("c5c")    # b1t2 chs 12:16 (last sliver)
    dsem = nc.alloc_semaphore("dsem")  # dum scratch initialized
    bsem = nc.alloc_semaphore("bsem")  # DVE qws1 builds
    pesem = nc.alloc_semaphore("pesem")  # PE per-tile accumulation stops
    nsem = nc.alloc_semaphore("nsem")    # DVE nrm writes
    scsem = nc.alloc_semaphore("scsem")  # Scalar exp (+den accum) done
    xsem = nc.alloc_semaphore("xsem")    # DVE scr write -> reduce RAW
    dvsem = nc.alloc_semaphore("dvsem")  # DVE per-tile epilogue complete
    osem = nc.alloc_semaphore("osem")    # output DMA completions

    TILES = [(b, t) for b in range(BPC) for t in range(3)]

    def ctp(b, t, lo, hi):
        return (cts[b][:, t, lo:hi, :], cT[b, :, t, lo:hi, :])

    with nc.Block(name="kern", no_gpsimd_drain=True) as blk:

        @blk.sync
        def _(eng):
            eng.dma_start(out=b2s[:, :, :], in_=bias2[:, :, :]) \
                .then_inc(gb, 16)
            eng.dma_start(out=qws[0][:, 0:8, :], in_=qwx0[:, 0:8, :]) \
                .then_inc(w0a, 16)
            for (o, i), sem in [
                (ctp(0, 0, 8, 16), cs[0][0][1]),
                (ctp(0, 1, 0, 8), cs[0][1][0]),
                (ctp(0, 2, 0, 8), cs[0][2][0]),
                (ctp(1, 0, 0, 8), cs[1][0][0]),
                (ctp(1, 1, 0, 8), cs[1][1][0]),
                (ctp(1, 2, 0, 8), cs[1][2][0]),
                (ctp(1, 2, 12, 16), c5c),
            ]:
                eng.dma_start(out=o, in_=i).then_inc(sem, 16)
            eng.wait_ge(dvsem, 3)
            eng.dma_start(out=outv[0, :, :],
                          in_=ovs[0].rearrange("p a b -> p (a b)")) \
                .then_inc(osem, 16)
            # batch 1 tiles 0-1 ship early from Scalar; only tile 2's 2KB
            # trails the last epilogue here
            eng.wait_ge(dvsem, 6)
            eng.dma_start(out=outv[1, :, 8:12],
                          in_=ovs[1][:, 2, :]) \
                .then_inc(osem, 16)
            eng.wait_ge(osem, 48)

        @blk.scalar
        def _(eng):
            eng.dma_start(out=wvs[:, :, :], in_=wv[:, :, :]) \
                .then_inc(gw, 16)
            eng.dma_start(out=qws[0][:, 8:16, :], in_=qwx0[:, 8:16, :]) \
                .then_inc(w0b, 16)
            eng.dma_start(out=qraw1[:, :, :], in_=qr1[:, :, :]) \
                .then_inc(q1s, 16)
            for (o, i), sem in [
                (ctp(0, 0, 0, 8), cs[0][0][0]),
                (ctp(0, 1, 8, 16), cs[0][1][1]),
                (ctp(0, 2, 8, 16), cs[0][2][1]),
                (ctp(1, 0, 8, 16), cs[1][0][1]),
                (ctp(1, 1, 8, 16), cs[1][1][1]),
                (ctp(1, 2, 8, 12), cs[1][2][1]),
            ]:
                eng.dma_start(out=o, in_=i).then_inc(sem, 16)
            for i, (b, t) in enumerate(TILES):
                eng.wait_ge(nsem, i + 1)
                eng.activation(es[:, i, :], ps[b][t][:, 0:Q], Exp,
                               bias=ovs[b][:, t, 2:3], scale=1.0,
                               accum_out=ovs[b][:, t, 3:4]) \
                    .then_inc(scsem, 1)
            eng.wait_ge(dvsem, 5)
            eng.dma_start(out=outv[1, :, 0:8],
                          in_=ovs[1][:, 0:2, :].rearrange("p a b -> p (a b)")) \
                .then_inc(osem, 16)

        @blk.tensor
        def _(eng):
            # warm-up matmuls on scratch data: keep the PE busy through
            # the DMA lead-in so the HAM clock-gate is at 2.4GHz when
            # the real matmuls start
            def dummy(n):
                for _ in range(n):
                    eng.matmul(pw[:, :], dum[:, 0, :], dum[:, 1, :],
                               start=True, stop=True)

            eng.wait_ge(dsem, 1)
            dummy(26)
            # bias matmuls OPEN each accumulation group (start=True) so
            # they run early, off the critical tail
            eng.wait_ge(gb, 16)
            for b, t in TILES:
                eng.matmul(ps[b][t][:, :],
                           b2s[:, b, NW + 128 * t:NW + 128 * (t + 1)],
                           b2s[:, b, 0:NW], start=True, stop=False)
            dummy(14)

            def grp(b, t, lo, hi, close, waits):
                for w in waits:
                    eng.wait_ge(w, 16)
                for ch in range(lo, hi):
                    mm = eng.matmul(ps[b][t][:, :],
                                    cts[b][:, t, ch, :],
                                    qws[b][:, ch, :],
                                    start=False, stop=(close and ch == hi - 1))
                    if close and ch == hi - 1:
                        mm.then_inc(pesem, 1)

            # piece order = expected DMA arrival order; each tile's
            # last-arriving piece closes its accumulation group
            grp(0, 0, 8, 16, False, [w0b, cs[0][0][1]])
            grp(0, 0, 0, 8, True, [w0a, cs[0][0][0]])
            grp(0, 1, 0, 8, False, [cs[0][1][0]])
            grp(0, 1, 8, 16, True, [cs[0][1][1]])
            grp(0, 2, 0, 8, False, [cs[0][2][0]])
            grp(0, 2, 8, 16, True, [cs[0][2][1]])
            eng.wait_ge(bsem, 3)
            grp(1, 0, 0, 8, False, [cs[1][0][0]])
            grp(1, 0, 8, 16, True, [cs[1][0][1]])
            grp(1, 1, 0, 8, False, [cs[1][1][0]])
            grp(1, 1, 8, 16, True, [cs[1][1][1]])
            grp(1, 2, 0, 8, False, [cs[1][2][0]])
            grp(1, 2, 8, 12, False, [cs[1][2][1]])
            grp(1, 2, 12, 16, True, [c5c])

        @blk.vector
        def _(eng):
            eng.memset(dum[:, :, :], 0.5).then_inc(dsem, 1)

            # build qws1[:, ch, 64k:64k+64] = qraw1[:, ch, :] * wv[:, ch, k]
            # (wv broadcast over the 64 q-cols via a stride-0 AP axis)
            eng.wait_ge(gw, 16)
            eng.wait_ge(q1s, 16)
            for k in range(3):
                wk = wvs[:, :, k:k + 1]
                wb = bass.AP(tensor=wk.tensor, offset=wk.offset,
                             ap=[wk.ap[0], [3, NCH], [0, Q]])
                eng.tensor_tensor(out=qws[1][:, :, 64 * k:64 * (k + 1)],
                                  in0=qraw1[:, :, :], in1=wb,
                                  op=mul_op).then_inc(bsem, 1)

            # software-pipelined epilogue: tile i+1's nrm issues before
            # tile i's mul/reduce so the Scalar exp overlaps the DVE work
            def nrm(i):
                b, t = TILES[i]
                eng.wait_ge(pesem, i + 1)
                eng.tensor_reduce(out=ovs[b][:, t, 2:3],
                                  in_=ps[b][t][:, 0:Q], axis=Ax,
                                  op=max_op, negate=True).then_inc(nsem, 1)

            def body(i):
                b, t = TILES[i]
                eng.wait_ge(scsem, i + 1)
                e = es[:, i, :]
                e_dup = bass.AP(tensor=e.tensor, offset=e.offset,
                                ap=[e.ap[0], [0, 2], e.ap[1]])
                eng.tensor_tensor(
                    out=scr[:, i, :, :],
                    in0=ps[b][t][:, Q:3 * Q].rearrange("p (j i) -> p j i",
                                                       j=2),
                    in1=e_dup, op=mul_op).then_inc(xsem, 1)
                eng.wait_ge(xsem, i + 1)
                eng.tensor_reduce(out=ovs[b][:, t, 0:2],
                                  in_=scr[:, i, :, :],
                                  axis=Ax, op=add_op).then_inc(dvsem, 1)

            nrm(0)
            for i in range(1, 6):
                nrm(i)
                body(i - 1)
            body(5)

    nc.finalize()
    return nc


def _get_nc():
    if "nc" not in _cache:
        _cache["nc"] = _build_nc()
    return _cache["nc"]


def _prep_host(c, q, c_len, q_len, w_c, b_c, w_q, b_q, w_cq, b_cq, W_out,
               b_out):
    """Build per-core device input maps (host-side layout/masking prep)."""
    c = np.asarray(c, np.float32)
    q = np.asarray(q, np.float32)
    c_len = np.asarray(c_len).astype(np.int64)
    q_len = np.asarray(q_len).astype(np.int64)
    w_c = np.asarray(w_c, np.float32)
    w_q = np.asarray(w_q, np.float32)
    w_cq = np.asarray(w_cq, np.float32)
    W_out = np.asarray(W_out, np.float32)
    b_out = np.asarray(b_out, np.float32)
    b_sum = float(np.asarray(b_c, np.float32) + np.asarray(b_q, np.float32)
                  + np.asarray(b_cq, np.float32))

    Mv = np.float32(BF16(-1e12))
    iq = np.arange(Q)
    W2 = W_out[D:2 * D]       # [D, 2] (x = [c, c2q, c*c2q, c*q2c])
    W3 = W_out[2 * D:3 * D]

    wvm = np.stack([w_cq, W3[:, 0], W3[:, 1]], axis=1) \
        .reshape(NCH, 128, 3).transpose(1, 0, 2).astype(BF16)

    in_maps = []
    for core in range(NCORES):
        bs = [BPC * core + i for i in range(BPC)]
        cTm = np.empty((BPC, 128, 3, NCH, 128), BF16)
        b2 = np.zeros((3, BPC, NW + C), BF16)
        for i, bidx in enumerate(bs):
            cTm[i] = c[bidx].T.reshape(NCH, 128, 3, 128) \
                .transpose(1, 2, 0, 3).astype(BF16)
            qb = q[bidx]
            qs = qb @ w_q + b_sum
            low = np.where(iq >= q_len[bidx], Mv, np.float32(0))
            hi = np.where((iq < Q - 1) | (iq >= q_len[bidx]), Mv,
                          np.float32(0))
            QW2b = qb @ W2 + b_out[None, :]
            b2[0, i, 0:64] = qs.astype(BF16)
            b2[0, i, 64:128] = QW2b[:, 0].astype(BF16)
            b2[0, i, 128:192] = QW2b[:, 1].astype(BF16)
            b2[1, i, 0:64] = low.astype(BF16)
            b2[2, i, 0:64] = (hi - low).astype(BF16)
            b2[0, i, NW:NW + C] = BF16(1)
            b2[1, i, NW:NW + C] = BF16(1)
            b2[2, i, NW:NW + C] = (np.arange(C) >= c_len[bidx]) \
                .astype(np.float32).astype(BF16)
        # batch 0: pre-scaled q-side rhs (3 copies)
        q0T = q[bs[0]].T                          # [D, Q]
        blk = np.empty((D, NW), np.float32)
        blk[:, 0:64] = q0T * w_cq[:, None]
        blk[:, 64:128] = q0T * W3[:, 0:1]
        blk[:, 128:192] = q0T * W3[:, 1:2]
        qwx0m = blk.reshape(NCH, 128, NW).transpose(1, 0, 2).astype(BF16)
        # batch 1: raw q, scaled on device
        qr1m = q[bs[1]].T.reshape(NCH, 128, Q).transpose(1, 0, 2) \
            .astype(BF16)
        in_maps.append(dict(cT=cTm, qwx0=qwx0m, qr1=qr1m, wv=wvm, bias2=b2))
    return in_maps, (c, c_len, W_out, w_c)


def kernel(**inputs):
    from concourse.bass_utils import run_bass_kernel_spmd

    nc = _get_nc()
    in_maps, (c, c_len, W_out, w_c) = _prep_host(**inputs)
    res = run_bass_kernel_spmd(nc, in_maps, core_ids=list(range(NCORES)))
    _cache["last_results"] = res

    W1 = W_out[0:D]          # [D, 2]
    W4 = W_out[3 * D:4 * D]

    out = np.empty((B, C, 2), np.float32)
    for core in range(NCORES):
        o = res.results[core]["outv"].reshape(BPC, 128, 3, 4)
        for i in range(BPC):
            bidx = BPC * core + i
            den = o[i, :, :, 3].T.reshape(C)
            t23 = o[i, :, :, 0:2].transpose(1, 0, 2).reshape(C, 2) \
                / den[:, None]
            nrm = o[i, :, :, 2].T.reshape(C)
            m = c[bidx] @ w_c - nrm
            eb = np.exp(m - m.max())
            b_att = (eb / eb.sum()).astype(np.float32)
            q2c = b_att @ c[bidx]                       # [D]
            w14 = W1 + W4 * q2c[:, None]                # [D, 2]
            out[bidx] = c[bidx] @ w14 + t23

    rows = np.arange(C)[None, :]
    row_mask = (rows >= c_len[:, None]) & (rows < C - 1)
    out0 = np.where(row_mask, NEG, out[..., 0])
    out1 = np.where(row_mask, NEG, out[..., 1])
    return out0, out1
